# revision 6
# baseline (speedup 1.0000x reference)
"""Cross-attention kernel for Trainium2, SPMD over 8 NeuronCores.

Problem: B=4, N=2048, C=1024 fp32.
  q = event_f @ Wq + bq ; k = img_f @ Wk + bk ; v = img_f @ Wv + bv
  out = softmax(q k^T / sqrt(C)) v

Sharding: core i = (batch b = i//2, query-half h = i%2). Each core computes
k/v for its full batch (duplicated across the pair) and attention for its
1024 query rows. No collectives.

Layout strategy (zero on-device transposes):
  Host ships event^T / img^T (feature-major) and Wq/Wk/Wv natural, all fp16.
  - q^T[c,nq]  = (Wq blk).T @ ev^T     (lhsT = Wq, rhs = ev^T)
  - k^T[c,nk]  = (Wk blk).T @ img^T
  - v[nk,c]    = (img^T blk).T @ Wv    (lhsT = img^T, rhs = Wv)
  - s^T[nk,nq] = (k^T blk).T @ q^T     (scores transposed: k on partitions)
  - p^T = exp(s^T * scale)             (no max-subtraction; logits are O(5))
  - out[nq,c]  = (p^T blk).T @ v       (p^T is the stationary operand)
  - sums[nq,1] = (p^T blk).T @ ones ; out *= 1/sums  (normalize at the end)
All matmul operands fp16, PSUM accumulation fp32, output fp32.
"""

import json

import numpy as np

B, N, C = 4, 2048, 1024
NQ = N // 2          # query rows per core
CT = C // 128        # contraction tiles
KT = N // 128        # key-row tiles
SCALE = 1.0 / np.sqrt(C)

_CACHE = {}


# ---------------------------------------------------------------------------
# Walrus in this container rejects >1 embedded sem-wait per instruction
# ("Too many sync wait commands"). Standalone waits are legal as
# EventSemaphore instructions, so hoist all but the last embedded wait.
def _fix_bir(bir: dict) -> dict:
    counter = [0]
    for fn in bir.get("functions", []):
        for bb in fn.get("blocks", []):
            out = []
            for ins in bb.get("instructions", []):
                si = ins.get("sync_info") or {}
                waits = si.get("on_wait") or []
                if len(waits) > 1 and ins.get("engine") not in (None, "Unassigned"):
                    for w in waits[:-1]:
                        counter[0] += 1
                        ev = {
                            "engine": ins["engine"],
                            "ins": [],
                            "name": f"hoistwait_{counter[0]}",
                            "opcode": "EventSemaphore",
                            "outs": [],
                            "sync_info": {"on_update": [], "on_wait": [w]},
                        }
                        if "debug" in ins:
                            ev["debug"] = ins["debug"]
                        out.append(ev)
                    si["on_wait"] = [waits[-1]]
                out.append(ins)
            bb["instructions"] = out
    return bir


def _install_waitfix(nc):
    orig = nc.to_json_bytes

    def patched():
        return json.dumps(_fix_bir(json.loads(orig()))).encode()

    nc.to_json_bytes = patched


# ---------------------------------------------------------------------------
def _build():
    import concourse.bass as bass
    import concourse.tile as tile
    from concourse import mybir

    f16, f32 = mybir.dt.float16, mybir.dt.float32
    Exp = mybir.ActivationFunctionType.Exp
    Ident = mybir.ActivationFunctionType.Identity

    nc = bass.Bass()
    ev_t = nc.dram_tensor("ev_t", [C, NQ], f16, kind="ExternalInput")
    img_t = nc.dram_tensor("img_t", [C, N], f16, kind="ExternalInput")
    wq = nc.dram_tensor("wq", [C, C], f16, kind="ExternalInput")
    wk = nc.dram_tensor("wk", [C, C], f16, kind="ExternalInput")
    wv = nc.dram_tensor("wv", [C, C], f16, kind="ExternalInput")
    bq = nc.dram_tensor("bq", [C], f32, kind="ExternalInput")
    bk = nc.dram_tensor("bk", [C], f32, kind="ExternalInput")
    bv = nc.dram_tensor("bv", [C], f32, kind="ExternalInput")
    out = nc.dram_tensor("out", [NQ, C], f32, kind="ExternalOutput")

    with tile.TileContext(nc) as tc:
        with (
            tc.tile_pool(name="ins", bufs=1) as ins_pool,
            tc.tile_pool(name="qkv", bufs=1) as qkv_pool,
            tc.tile_pool(name="expp", bufs=1) as exp_pool,
            tc.tile_pool(name="work", bufs=4) as work,
            tc.tile_pool(name="ps_proj", bufs=2, space="PSUM") as pp_proj,
            tc.tile_pool(name="ps_sc", bufs=2, space="PSUM") as pp_sc,
            tc.tile_pool(name="ps_out", bufs=2, space="PSUM") as pp_out,
            tc.tile_pool(name="ps_sum", bufs=2, space="PSUM") as pp_sum,
        ):
            # ---- stage A: inputs to SBUF --------------------------------
            # Per-c_in-tile DMAs so the first accumulation groups can start
            # while later tiles are still in flight (parallel DMA queues).
            ev_sb = ins_pool.tile([128, CT, NQ], f16)
            wq_sb = ins_pool.tile([128, CT, C], f16)
            img_sb = ins_pool.tile([128, CT, N], f16)
            wk_sb = ins_pool.tile([128, CT, C], f16)
            wv_sb = ins_pool.tile([128, CT, C], f16)
            ev_r = ev_t.rearrange("(t p) n -> t p n", p=128)
            wq_r = wq.rearrange("(t p) n -> t p n", p=128)
            img_r = img_t.rearrange("(t p) n -> t p n", p=128)
            wk_r = wk.rearrange("(t p) n -> t p n", p=128)
            wv_r = wv.rearrange("(t p) n -> t p n", p=128)
            for t in range(CT):
                nc.sync.dma_start(out=wq_sb[:, t, :], in_=wq_r[t])
                nc.sync.dma_start(out=ev_sb[:, t, :], in_=ev_r[t])
            for t in range(CT):
                nc.sync.dma_start(out=img_sb[:, t, :], in_=img_r[t])
                nc.sync.dma_start(out=wk_sb[:, t, :], in_=wk_r[t])
                nc.sync.dma_start(out=wv_sb[:, t, :], in_=wv_r[t])

            bq_sb = ins_pool.tile([128, CT], f32)
            bk_sb = ins_pool.tile([128, CT], f32)
            nc.sync.dma_start(out=bq_sb[:], in_=bq.rearrange("(t p) -> p t", p=128))
            nc.sync.dma_start(out=bk_sb[:], in_=bk.rearrange("(t p) -> p t", p=128))
            # v-bias varies along the free dim -> broadcast row to 128 parts
            bv_sb = ins_pool.tile([128, C], f32)
            nc.sync.dma_start(out=bv_sb[:], in_=bv[None, :].to_broadcast((128, C)))
            ones_sb = ins_pool.tile([128, 1], f16)
            nc.vector.memset(ones_sb[:], 1.0)

            # ---- stage B: projections -----------------------------------
            q_sb = qkv_pool.tile([128, CT, NQ], f16)   # q^T  [c | nq]
            k_sb = qkv_pool.tile([128, CT, N], f16)    # k^T  [c | nk]
            v_sb = qkv_pool.tile([128, KT, C], f16)    # v    [nk | c]

            # q^T and k^T: per c_out tile, chunks of 512 over rows
            for co in range(CT):
                for ch in range(NQ // 512):
                    ps = pp_proj.tile([128, 512], f32, tag="ps_proj")
                    for t in range(CT):
                        nc.tensor.matmul(
                            ps,
                            wq_sb[:, t, co * 128:(co + 1) * 128],
                            ev_sb[:, t, ch * 512:(ch + 1) * 512],
                            start=(t == 0), stop=(t == CT - 1),
                        )
                    nc.scalar.activation(
                        q_sb[:, co, ch * 512:(ch + 1) * 512], ps, Ident,
                        bias=bq_sb[:, co:co + 1],
                    )
            for co in range(CT):
                for ch in range(N // 512):
                    ps = pp_proj.tile([128, 512], f32, tag="ps_proj")
                    for t in range(CT):
                        nc.tensor.matmul(
                            ps,
                            wk_sb[:, t, co * 128:(co + 1) * 128],
                            img_sb[:, t, ch * 512:(ch + 1) * 512],
                            start=(t == 0), stop=(t == CT - 1),
                        )
                    nc.scalar.activation(
                        k_sb[:, co, ch * 512:(ch + 1) * 512], ps, Ident,
                        bias=bk_sb[:, co:co + 1],
                    )
            # v: natural layout, bias added along free dim on DVE
            for nr in range(KT):
                for ch in range(C // 512):
                    ps = pp_proj.tile([128, 512], f32, tag="ps_proj")
                    for t in range(CT):
                        nc.tensor.matmul(
                            ps,
                            img_sb[:, t, nr * 128:(nr + 1) * 128],
                            wv_sb[:, t, ch * 512:(ch + 1) * 512],
                            start=(t == 0), stop=(t == CT - 1),
                        )
                    nc.vector.tensor_add(
                        v_sb[:, nr, ch * 512:(ch + 1) * 512], ps,
                        bv_sb[:, ch * 512:(ch + 1) * 512],
                    )

            # ---- stage C: attention, 512 query rows at a time -----------
            for qc in range(NQ // 512):
                exp_sb = exp_pool.tile([128, KT, 512], f16, tag="exp")
                for kt in range(KT):
                    ps = pp_sc.tile([128, 512], f32, tag="ps_sc")
                    for t in range(CT):
                        nc.tensor.matmul(
                            ps,
                            k_sb[:, t, kt * 128:(kt + 1) * 128],
                            q_sb[:, t, qc * 512:(qc + 1) * 512],
                            start=(t == 0), stop=(t == CT - 1),
                        )
                    nc.scalar.activation(exp_sb[:, kt, :], ps, Exp, scale=float(SCALE))

                for q4 in range(4):
                    qlo = q4 * 128
                    sums = pp_sum.tile([128, 1], f32, tag="ps_sum")
                    for kt in range(KT):
                        nc.tensor.matmul(
                            sums,
                            exp_sb[:, kt, qlo:qlo + 128],
                            ones_sb[:],
                            start=(kt == 0), stop=(kt == KT - 1),
                        )
                    recip = work.tile([128, 1], f32, tag="recip")
                    nc.vector.reciprocal(recip[:], sums)

                    for ch in range(C // 512):
                        ps = pp_out.tile([128, 512], f32, tag="ps_out")
                        for kt in range(KT):
                            nc.tensor.matmul(
                                ps,
                                exp_sb[:, kt, qlo:qlo + 128],
                                v_sb[:, kt, ch * 512:(ch + 1) * 512],
                                start=(kt == 0), stop=(kt == KT - 1),
                            )
                        o_sb = work.tile([128, 512], f32, tag="o")
                        nc.vector.tensor_scalar_mul(o_sb[:], ps, recip[:])
                        nc.sync.dma_start(
                            out=out[qc * 512 + qlo:qc * 512 + qlo + 128,
                                    ch * 512:(ch + 1) * 512],
                            in_=o_sb[:],
                        )
    _install_waitfix(nc)
    return nc


def _get_nc():
    if "nc" not in _CACHE:
        _CACHE["nc"] = _build()
    return _CACHE["nc"]


def run(inputs, trace=False, trace_cores=None):
    from concourse.bass_utils import run_bass_kernel_spmd

    event_f = np.asarray(inputs["event_f"], dtype=np.float32)
    img_f = np.asarray(inputs["img_f"], dtype=np.float32)
    Wq = np.asarray(inputs["Wq"], dtype=np.float32).astype(np.float16)
    Wk = np.asarray(inputs["Wk"], dtype=np.float32).astype(np.float16)
    Wv = np.asarray(inputs["Wv"], dtype=np.float32).astype(np.float16)
    bq = np.asarray(inputs["bq"], dtype=np.float32)
    bk = np.asarray(inputs["bk"], dtype=np.float32)
    bv = np.asarray(inputs["bv"], dtype=np.float32)

    in_maps = []
    for core in range(8):
        b, h = core // 2, core % 2
        ev_t = np.ascontiguousarray(
            event_f[b, h * NQ:(h + 1) * NQ, :].T.astype(np.float16))
        img_t = np.ascontiguousarray(img_f[b].T.astype(np.float16))
        in_maps.append({
            "ev_t": ev_t, "img_t": img_t,
            "wq": Wq, "wk": Wk, "wv": Wv,
            "bq": bq, "bk": bk, "bv": bv,
        })

    nc = _get_nc()
    res = run_bass_kernel_spmd(
        nc, in_maps, list(range(8)), trace=trace,
        **({"trace_cores": trace_cores} if trace_cores else {}),
    )
    full = np.empty((B, N, C), dtype=np.float32)
    for core in range(8):
        b, h = core // 2, core % 2
        full[b, h * NQ:(h + 1) * NQ, :] = res.results[core]["out"]
    return full, res


def kernel(**inputs) -> np.ndarray:
    full, _ = run(inputs, trace=False)
    return full


# revision 9
# speedup vs baseline: 1.0199x; 1.0199x over previous
"""Cross-attention kernel for Trainium2, SPMD over 8 NeuronCores.

Problem: B=4, N=2048, C=1024 fp32.
  q = event_f @ Wq + bq ; k = img_f @ Wk + bk ; v = img_f @ Wv + bv
  out = softmax(q k^T / sqrt(C)) v

Sharding: core i = (batch b = i//2, query-half h = i%2). Each core computes
k/v for its full batch (duplicated across the pair) and attention for its
1024 query rows. No collectives.

Layout strategy (zero on-device transposes):
  Host ships event^T / img^T (feature-major) and Wq/Wk/Wv natural, all fp16.
  - q^T[c,nq]  = (Wq blk).T @ ev^T     (lhsT = Wq, rhs = ev^T)
  - k^T[c,nk]  = (Wk blk).T @ img^T
  - v[nk,c]    = (img^T blk).T @ Wv    (lhsT = img^T, rhs = Wv)
  - s^T[nk,nq] = (k^T blk).T @ q^T     (scores transposed: k on partitions)
  - p^T = exp(s^T * scale)             (no max-subtraction; logits are O(5))
  - out[nq,c]  = (p^T blk).T @ v       (p^T is the stationary operand)
  - sums[nq,1] = (p^T blk).T @ ones ; out *= 1/sums  (normalize at the end)
All matmul operands fp16, PSUM accumulation fp32, output fp32.
"""

import json

import numpy as np

B, N, C = 4, 2048, 1024
NQ = N // 2          # query rows per core
CT = C // 128        # contraction tiles
KT = N // 128        # key-row tiles
SCALE = 1.0 / np.sqrt(C)

_CACHE = {}


# ---------------------------------------------------------------------------
# Walrus in this container rejects >1 embedded sem-wait per instruction
# ("Too many sync wait commands"). Standalone waits are legal as
# EventSemaphore instructions, so hoist all but the last embedded wait.
def _fix_bir(bir: dict) -> dict:
    counter = [0]
    for fn in bir.get("functions", []):
        for bb in fn.get("blocks", []):
            out = []
            for ins in bb.get("instructions", []):
                si = ins.get("sync_info") or {}
                waits = si.get("on_wait") or []
                if len(waits) > 1 and ins.get("engine") not in (None, "Unassigned"):
                    for w in waits[:-1]:
                        counter[0] += 1
                        ev = {
                            "engine": ins["engine"],
                            "ins": [],
                            "name": f"hoistwait_{counter[0]}",
                            "opcode": "EventSemaphore",
                            "outs": [],
                            "sync_info": {"on_update": [], "on_wait": [w]},
                        }
                        if "debug" in ins:
                            ev["debug"] = ins["debug"]
                        out.append(ev)
                    si["on_wait"] = [waits[-1]]
                out.append(ins)
            bb["instructions"] = out
    return bir


def _install_waitfix(nc):
    orig = nc.to_json_bytes

    def patched():
        return json.dumps(_fix_bir(json.loads(orig()))).encode()

    nc.to_json_bytes = patched


# ---------------------------------------------------------------------------
def _build():
    import concourse.bass as bass
    import concourse.tile as tile
    from concourse import mybir

    f16, f32 = mybir.dt.float16, mybir.dt.float32
    Exp = mybir.ActivationFunctionType.Exp
    Ident = mybir.ActivationFunctionType.Identity

    nc = bass.Bass()
    ev_t = nc.dram_tensor("ev_t", [C, NQ], f16, kind="ExternalInput")
    img_t = nc.dram_tensor("img_t", [C, N], f16, kind="ExternalInput")
    wq = nc.dram_tensor("wq", [C, C], f16, kind="ExternalInput")
    wk = nc.dram_tensor("wk", [C, C], f16, kind="ExternalInput")
    wv = nc.dram_tensor("wv", [C, C], f16, kind="ExternalInput")
    bq = nc.dram_tensor("bq", [C], f32, kind="ExternalInput")
    bk = nc.dram_tensor("bk", [C], f32, kind="ExternalInput")
    bv = nc.dram_tensor("bv", [C], f32, kind="ExternalInput")
    out = nc.dram_tensor("out", [NQ, C], f32, kind="ExternalOutput")

    with tile.TileContext(nc) as tc:
        with (
            tc.tile_pool(name="ins", bufs=1) as ins_pool,
            tc.tile_pool(name="qkv", bufs=1) as qkv_pool,
            tc.tile_pool(name="expp", bufs=1) as exp_pool,
            tc.tile_pool(name="work", bufs=4) as work,
            tc.tile_pool(name="ps_proj", bufs=2, space="PSUM") as pp_proj,
            tc.tile_pool(name="ps_sc", bufs=2, space="PSUM") as pp_sc,
            tc.tile_pool(name="ps_out", bufs=2, space="PSUM") as pp_out,
            tc.tile_pool(name="ps_sum", bufs=2, space="PSUM") as pp_sum,
        ):
            # ---- stage A: inputs to SBUF --------------------------------
            # Per-c_in-tile DMAs so the first accumulation groups can start
            # while later tiles are still in flight (parallel DMA queues).
            ev_r = ev_t.rearrange("(t p) n -> t p n", p=128)
            wq_r = wq.rearrange("(t p) n -> t p n", p=128)
            img_r = img_t.rearrange("(t p) n -> t p n", p=128)
            wk_r = wk.rearrange("(t p) n -> t p n", p=128)
            wv_r = wv.rearrange("(t p) n -> t p n", p=128)
            ev_sb, wq_sb, img_sb, wk_sb, wv_sb = [], [], [], [], []
            for t in range(CT):
                wq_sb.append(ins_pool.tile([128, C], f16, name=f"wq{t}", tag=f"wq{t}"))
                nc.sync.dma_start(out=wq_sb[t][:], in_=wq_r[t])
                ev_sb.append(ins_pool.tile([128, NQ], f16, name=f"ev{t}", tag=f"ev{t}"))
                nc.sync.dma_start(out=ev_sb[t][:], in_=ev_r[t])
            for t in range(CT):
                img_sb.append(ins_pool.tile([128, N], f16, name=f"img{t}", tag=f"img{t}"))
                nc.sync.dma_start(out=img_sb[t][:], in_=img_r[t])
                wk_sb.append(ins_pool.tile([128, C], f16, name=f"wk{t}", tag=f"wk{t}"))
                nc.sync.dma_start(out=wk_sb[t][:], in_=wk_r[t])
                wv_sb.append(ins_pool.tile([128, C], f16, name=f"wv{t}", tag=f"wv{t}"))
                nc.sync.dma_start(out=wv_sb[t][:], in_=wv_r[t])

            bq_sb = ins_pool.tile([128, CT], f32)
            bk_sb = ins_pool.tile([128, CT], f32)
            nc.sync.dma_start(out=bq_sb[:], in_=bq.rearrange("(t p) -> p t", p=128))
            nc.sync.dma_start(out=bk_sb[:], in_=bk.rearrange("(t p) -> p t", p=128))
            # v-bias varies along the free dim -> broadcast row to 128 parts
            bv_sb = ins_pool.tile([128, C], f32)
            nc.sync.dma_start(out=bv_sb[:], in_=bv[None, :].to_broadcast((128, C)))
            ones_sb = ins_pool.tile([128, 1], f16)
            nc.vector.memset(ones_sb[:], 1.0)

            # ---- stage B: projections -----------------------------------
            q_sb = qkv_pool.tile([128, CT, NQ], f16)   # q^T  [c | nq]
            k_sb = qkv_pool.tile([128, CT, N], f16)    # k^T  [c | nk]
            v_sb = qkv_pool.tile([128, KT, C], f16)    # v    [nk | c]

            # q^T and k^T: per c_out tile, chunks of 512 over rows
            for co in range(CT):
                for ch in range(NQ // 512):
                    ps = pp_proj.tile([128, 512], f32, tag="ps_proj")
                    for t in range(CT):
                        nc.tensor.matmul(
                            ps,
                            wq_sb[t][:, co * 128:(co + 1) * 128],
                            ev_sb[t][:, ch * 512:(ch + 1) * 512],
                            start=(t == 0), stop=(t == CT - 1),
                        )
                    nc.scalar.activation(
                        q_sb[:, co, ch * 512:(ch + 1) * 512], ps, Ident,
                        bias=bq_sb[:, co:co + 1],
                    )
            for co in range(CT):
                for ch in range(N // 512):
                    ps = pp_proj.tile([128, 512], f32, tag="ps_proj")
                    for t in range(CT):
                        nc.tensor.matmul(
                            ps,
                            wk_sb[t][:, co * 128:(co + 1) * 128],
                            img_sb[t][:, ch * 512:(ch + 1) * 512],
                            start=(t == 0), stop=(t == CT - 1),
                        )
                    nc.scalar.activation(
                        k_sb[:, co, ch * 512:(ch + 1) * 512], ps, Ident,
                        bias=bk_sb[:, co:co + 1],
                    )
            # v: natural layout, bias added along free dim on DVE
            for nr in range(KT):
                for ch in range(C // 512):
                    ps = pp_proj.tile([128, 512], f32, tag="ps_proj")
                    for t in range(CT):
                        nc.tensor.matmul(
                            ps,
                            img_sb[t][:, nr * 128:(nr + 1) * 128],
                            wv_sb[t][:, ch * 512:(ch + 1) * 512],
                            start=(t == 0), stop=(t == CT - 1),
                        )
                    nc.vector.tensor_add(
                        v_sb[:, nr, ch * 512:(ch + 1) * 512], ps,
                        bv_sb[:, ch * 512:(ch + 1) * 512],
                    )

            # ---- stage C: attention, 512 query rows at a time -----------
            for qc in range(NQ // 512):
                exp_sb = exp_pool.tile([128, KT, 512], f16, tag="exp")
                for kt in range(KT):
                    ps = pp_sc.tile([128, 512], f32, tag="ps_sc")
                    for t in range(CT):
                        nc.tensor.matmul(
                            ps,
                            k_sb[:, t, kt * 128:(kt + 1) * 128],
                            q_sb[:, t, qc * 512:(qc + 1) * 512],
                            start=(t == 0), stop=(t == CT - 1),
                        )
                    nc.scalar.activation(exp_sb[:, kt, :], ps, Exp, scale=float(SCALE))

                for q4 in range(4):
                    qlo = q4 * 128
                    sums = pp_sum.tile([128, 1], f32, tag="ps_sum")
                    for kt in range(KT):
                        nc.tensor.matmul(
                            sums,
                            exp_sb[:, kt, qlo:qlo + 128],
                            ones_sb[:],
                            start=(kt == 0), stop=(kt == KT - 1),
                        )
                    recip = work.tile([128, 1], f32, tag="recip")
                    nc.vector.reciprocal(recip[:], sums)

                    for ch in range(C // 512):
                        ps = pp_out.tile([128, 512], f32, tag="ps_out")
                        for kt in range(KT):
                            nc.tensor.matmul(
                                ps,
                                exp_sb[:, kt, qlo:qlo + 128],
                                v_sb[:, kt, ch * 512:(ch + 1) * 512],
                                start=(kt == 0), stop=(kt == KT - 1),
                            )
                        o_sb = work.tile([128, 512], f32, tag="o")
                        nc.vector.tensor_scalar_mul(o_sb[:], ps, recip[:])
                        nc.sync.dma_start(
                            out=out[qc * 512 + qlo:qc * 512 + qlo + 128,
                                    ch * 512:(ch + 1) * 512],
                            in_=o_sb[:],
                        )
    _install_waitfix(nc)
    return nc


def _get_nc():
    if "nc" not in _CACHE:
        _CACHE["nc"] = _build()
    return _CACHE["nc"]


def run(inputs, trace=False, trace_cores=None):
    from concourse.bass_utils import run_bass_kernel_spmd

    event_f = np.asarray(inputs["event_f"], dtype=np.float32)
    img_f = np.asarray(inputs["img_f"], dtype=np.float32)
    Wq = np.asarray(inputs["Wq"], dtype=np.float32).astype(np.float16)
    Wk = np.asarray(inputs["Wk"], dtype=np.float32).astype(np.float16)
    Wv = np.asarray(inputs["Wv"], dtype=np.float32).astype(np.float16)
    bq = np.asarray(inputs["bq"], dtype=np.float32)
    bk = np.asarray(inputs["bk"], dtype=np.float32)
    bv = np.asarray(inputs["bv"], dtype=np.float32)

    in_maps = []
    for core in range(8):
        b, h = core // 2, core % 2
        ev_t = np.ascontiguousarray(
            event_f[b, h * NQ:(h + 1) * NQ, :].T.astype(np.float16))
        img_t = np.ascontiguousarray(img_f[b].T.astype(np.float16))
        in_maps.append({
            "ev_t": ev_t, "img_t": img_t,
            "wq": Wq, "wk": Wk, "wv": Wv,
            "bq": bq, "bk": bk, "bv": bv,
        })

    nc = _get_nc()
    res = run_bass_kernel_spmd(
        nc, in_maps, list(range(8)), trace=trace,
        **({"trace_cores": trace_cores} if trace_cores else {}),
    )
    full = np.empty((B, N, C), dtype=np.float32)
    for core in range(8):
        b, h = core // 2, core % 2
        full[b, h * NQ:(h + 1) * NQ, :] = res.results[core]["out"]
    return full, res


def kernel(**inputs) -> np.ndarray:
    full, _ = run(inputs, trace=False)
    return full
